# revision 40
# baseline (speedup 1.0000x reference)
"""Trainium2 Bass kernel for nn_EternalNeuralLayer.

Math: out = tanh(x @ W_c + b_c + probs[None, :]) where
probs[j] = |state[j, 0]|^2 after 27 nearest-neighbour circulant "gates"
applied to the uniform state 1/sqrt(n). Each gate matrix
G = cos*I - sin*P + sin*P^T is circulant, and the uniform vector is its
eigenvector with eigenvalue cos(theta), so the state stays uniform:
probs[j] = (prod_{d,g} cos(ew[d, j, g]))^2 / n   (g in 0..2, d in 0..8).

Sharding: data-parallel over the batch (8 cores x 512 rows). Every core
streams the full classical_weights [2048, 2048], computes its x-shard's
GEMM as outT[m, b] = sum_k W[k, m] * xT[k, b] (output m-on-partitions),
and writes its outT shard. The eternal-probs product is computed
on-device per core from the [27, 2048] angle slice (tiny). No
collectives needed.

GEMM precision scheme (~16 effective mantissa bits, abs err ~2e-3):
  z * 2^11 = (xh*2^5) @ (Wh*2^6)  fp16 x fp16 matmul   (1.0 cyc/row)
           + xl8 @ W8             fp8e4 DoubleRow      (0.5 cyc/row)
           + x8  @ Wl8            fp8e4 DoubleRow      (0.5 cyc/row)
  xh = fp16(x), xl8 = e4m3((x - xh) * 2^11), x8 = e4m3(x); same for W.
The fp16 main pass carries 11+11 bits; the two fp8 DoubleRow passes
(2 k-rows/cycle) correct each operand's fp16 rounding to ~16+16 bits.
Cross-term quantization error ~9e-4 rms on the preactivation. This
replaces the 3x fp32r hi/lo scheme: 2.0 pass-equivalents instead of
3.0, and 25MB HBM traffic instead of 46MB per core.

The power-of-2 operand prescale puts the fp16 main pass at the same
2^11 fixed-point scale the fp8 correction planes already carry, so all
three passes accumulate into ONE PSUM bank and the epilogue is a single
Scalar-engine tanh(psum * 2^-11 + bias) per m-tile.
"""

import math
import os
import sys

import numpy as np

for _p in ("/opt/trn_rl_repo", "/root/.axon_site/_ro/trn_rl_repo"):
    if _p not in sys.path and os.path.isdir(_p):
        sys.path.append(_p)

import concourse.bass as bass  # noqa: E402
import concourse.tile as tile  # noqa: E402
from concourse import bacc, mybir  # noqa: E402
from concourse.bass_utils import run_bass_kernel_spmd  # noqa: E402

N_CORES = 8
B, N, M, D = 4096, 2048, 2048, 9
BS = B // N_CORES  # 512 batch rows per core
KT = N // 128  # 16 contraction tiles
MT = M // 128  # 16 output m-tiles
WPRE = 8  # W-tile pool depth (per plane: one 8-tile phase in flight)
NGATE = D * 3  # 27 rotation gates
GPAD = 32  # padded gate slots (pad with 0.0 -> cos = 1)
CSH = 11  # correction planes carry 2^CSH fixed-point scale
CSCALE = float(2.0**CSH)

F32 = mybir.dt.float32
F16 = mybir.dt.float16
F8 = mybir.dt.float8e4
NP_F8 = mybir.dt.np(mybir.dt.float8e4)  # ml_dtypes.float8_e4m3

XC = 2  # xh DMA chunk: 2 k-slabs (2KB rows)
X8C = 4  # x8/xl8 DMA chunk: 4 k-slabs (2KB rows)


def build_program():
    nc = bacc.Bacc(
        "TRN2", target_bir_lowering=False, debug=False, num_devices=N_CORES
    )
    # x planes: [p, kb*BS + b] = plane[b, kb*128 + p]
    xh_d = nc.dram_tensor("xh", [128, KT * BS], F16, kind="ExternalInput").ap()
    x8_d = nc.dram_tensor("x8", [128, KT * BS], F8, kind="ExternalInput").ap()
    xl8_d = nc.dram_tensor("xl8", [128, KT * BS], F8, kind="ExternalInput").ap()
    # W planes: [t*128 + p, kb*128 + m] = Wplane[kb*128 + p, t*128 + m]
    wh_d = nc.dram_tensor("wh", [M, N], F16, kind="ExternalInput").ap()
    w8_d = nc.dram_tensor("w8", [M, N], F8, kind="ExternalInput").ap()
    wl8_d = nc.dram_tensor("wl8", [M, N], F8, kind="ExternalInput").ap()
    ang_d = nc.dram_tensor("ang", [128, GPAD * MT], F32, kind="ExternalInput").ap()
    cbt_d = nc.dram_tensor("cbt", [128, MT], F32, kind="ExternalInput").ap()
    # out_dev[t, ml, b] = tanh(...)[m = t*128 + ml, b]; bf16 output
    # (tanh is in [-1,1]: <=2^-9 quantization, well inside the budget)
    out_d = nc.dram_tensor(
        "out_dev", [MT, 128, BS], mybir.dt.bfloat16, kind="ExternalOutput"
    ).ap()

    with tile.TileContext(nc) as tc:
        with (
            tc.tile_pool(name="xt", bufs=1) as xt_pool,
            tc.tile_pool(name="w", bufs=WPRE) as w_pool,
            tc.tile_pool(name="ps", bufs=3, space="PSUM") as ps_pool,
            tc.tile_pool(name="out", bufs=3) as out_pool,
            tc.tile_pool(name="small", bufs=1) as small_pool,
        ):
            # --- resident x planes (filled by chunked DMAs; chunk sizes
            # keep every DMA row >= 2KB contiguous) ---
            xh_t = xt_pool.tile([128, KT, BS], F16, tag="xh")
            x8_t = xt_pool.tile([128, KT, BS], F8, tag="x8")
            xl8_t = xt_pool.tile([128, KT, BS], F8, tag="xl8")

            def fetch_xh_slabs(lo, hi):  # k-slab range [lo, hi)
                nc.sync.dma_start(
                    xh_t[:, lo:hi, :], xh_d[:, lo * BS : hi * BS]
                )

            def fetch_xh(c):  # chunk of XC k-slabs
                fetch_xh_slabs(c * XC, (c + 1) * XC)

            def fetch_x8(c, t, d):  # chunk of X8C k-slabs
                nc.sync.dma_start(
                    t[:, c * X8C : (c + 1) * X8C, :],
                    d[:, c * X8C * BS : (c + 1) * X8C * BS],
                )

            # --- GEMM input DMAs on the sync HWDGE ring; output stores use
            # the scalar ring so a store waiting on ACT never head-of-line
            # blocks W loads. ---
            wts = {}

            def w_plane_parts(t, plane, spans=((0, KT),)):
                """Allocate the plane tile, return per-span DMA thunks
                (spans in k-slab units)."""
                tag, dt_, d = (
                    ("wh", F16, wh_d), ("w8", F8, w8_d), ("wl8", F8, wl8_d)
                )[plane]
                if plane == 0:
                    wt = w_pool.tile([128, KT * 128], dt_, name="wt", tag=tag)
                else:
                    wt = w_pool.tile([128, KT, 128], dt_, name="wt", tag=tag)
                rows = slice(t * 128, (t + 1) * 128)
                wts.setdefault(t, []).append(wt)

                def issue(lo, hi):
                    nc.sync.dma_start(
                        wt[:, lo:hi, :]
                        if plane else wt[:, lo * 128 : hi * 128],
                        d[rows, lo * 128 : hi * 128],
                    )
                return [lambda lo=lo, hi=hi: issue(lo, hi) for lo, hi in spans]

            def fetch_w_plane(t, plane):
                w_plane_parts(t, plane)[0]()

            # --- eternal probs bias inputs ride the (idle until stores)
            # scalar ring so bias_t is ready long before the first epilogue
            ang = small_pool.tile([128, GPAD * MT], F32)
            nc.scalar.dma_start(ang[:], ang_d[:])
            cbt = small_pool.tile([128, MT], F32)
            nc.scalar.dma_start(cbt[:], cbt_d[:])

            # Issue order. Sync-queue DMA issue costs ~650ns/descriptor, so
            # the head uses few, ~0.25-0.5MB descriptors, ordered to match
            # the PE consumption schedule (mains 0-3, then corr0 lagging):
            # xh chunks with wh0's two halves woven in, wh1-3, the fp8
            # planes for the first corrs, then the steady pipeline.
            # ~0.25MB head descriptors: finer slicing starts the PE earlier
            # but into an unramped clock + stop-start stream and measures
            # net-worse; this pacing keeps the PE stall-free from its start
            wh0_parts = w_plane_parts(0, 0, spans=((0, 8), (8, 16)))
            fetch_xh_slabs(0, 2)
            wh0_parts[0]()
            fetch_xh(1)
            fetch_xh(2)
            wh0_parts[1]()
            for c in range(3, KT // XC):
                fetch_xh(c)
            fetch_w_plane(1, 0)
            fetch_w_plane(2, 0)
            fetch_w_plane(3, 0)
            for c in range(KT // X8C):
                fetch_x8(c, xl8_t, xl8_d)
            fetch_w_plane(0, 1)
            fetch_w_plane(0, 2)
            for c in range(KT // X8C):
                fetch_x8(c, x8_t, x8_d)
            for t in range(4, 8):
                fetch_w_plane(t, 0)
            for t in range(1, 8):
                fetch_w_plane(t, 1)
                fetch_w_plane(t, 2)

            # explicit zeros AP as ACT bias: a float bias would materialize
            # a const AP, whose TENSOR_LOAD serializes into the preamble
            zeros = small_pool.tile([128, 1], F32)
            nc.gpsimd.memset(zeros[:], 0.0)

            cosa = small_pool.tile([128, GPAD * MT], F32)
            # cos(a) = sin(a + pi/2); wrap into ACT Sin's [-pi, pi] domain
            # (|a| < 3pi/2 + pi holds for randn angles).
            nc.vector.add_range_wrap(
                cosa[:], ang[:], shift=math.pi / 2, bound=math.pi,
                period=2 * math.pi,
            )
            nc.scalar.activation(
                cosa[:], cosa[:], mybir.ActivationFunctionType.Sin,
                bias=zeros[:],
            )
            # tree-product over the 32 gate slots -> [128, MT]
            half = GPAD * MT // 2
            while half >= MT:
                nc.vector.tensor_mul(
                    cosa[:, 0:half], cosa[:, 0:half], cosa[:, half : 2 * half]
                )
                half //= 2
            bias_t = small_pool.tile([128, MT], F32)
            # probs = (prod cos)^2 / n
            nc.scalar.activation(
                bias_t[:],
                cosa[:, 0:MT],
                mybir.ActivationFunctionType.Square,
                bias=zeros[:],
                scale=1.0 / math.sqrt(N),
            )
            nc.vector.tensor_add(bias_t[:], bias_t[:], cbt[:])

            # --- column-parallel GEMM over 16 m-tiles ---
            def epilogue(t, ps, last=False):
                ot = out_pool.tile([128, BS], mybir.dt.bfloat16, name="ot", tag="ot", bufs=4)
                # out = tanh(ps * 2^-CSH + bias)  (single ACT pass)
                if not last:
                    nc.scalar.activation(
                        ot[:],
                        ps[:],
                        mybir.ActivationFunctionType.Tanh,
                        bias=bias_t[:, t : t + 1],
                        scale=1.0 / CSCALE,
                    )
                    nc.scalar.dma_start(out_d[t], ot[:])
                    return
                # final tile: halves, stores split across both rings, so
                # the exposed tail is ACT/2 + store/2 instead of ACT+store
                h = BS // 2
                nc.scalar.activation(
                    ot[:, 0:h], ps[:, 0:h],
                    mybir.ActivationFunctionType.Tanh,
                    bias=bias_t[:, t : t + 1], scale=1.0 / CSCALE,
                )
                nc.sync.dma_start(out_d[t][:, 0:h], ot[:, 0:h])
                nc.scalar.activation(
                    ot[:, h:BS], ps[:, h:BS],
                    mybir.ActivationFunctionType.Tanh,
                    bias=bias_t[:, t : t + 1], scale=1.0 / CSCALE,
                )
                nc.scalar.dma_start(out_d[t][:, h:BS], ot[:, h:BS])

            def main_chain(t):
                # fp16 main pass (prescaled by 2^11 across the operands)
                wh_w = wts[t][0]
                ps = ps_pool.tile([128, BS], F32, name="ps", tag="ps", bufs=8)
                pss[t] = ps
                for kb in range(KT):
                    nc.tensor.matmul(
                        ps[:],
                        lhsT=wh_w[:, kb * 128 : (kb + 1) * 128],
                        rhs=xh_t[:, kb, :],
                        start=(kb == 0), stop=False,
                    )

            def corr_chain(t):
                # fp8 DoubleRow corrections, 2 k-slabs per matmul, same
                # 2^11 scale -> same PSUM accumulator as the main pass
                _, w8_w, wl8_w = wts.pop(t)
                ps = pss.pop(t)
                for j in range(KT // 2):
                    nc.tensor.matmul(
                        ps[:],
                        lhsT=w8_w[:, 2 * j : 2 * j + 2, :],
                        rhs=xl8_t[:, 2 * j : 2 * j + 2, :],
                        start=False, stop=False,
                        perf_mode=mybir.MatmulPerfMode.DoubleRow,
                    )
                for j in range(KT // 2):
                    nc.tensor.matmul(
                        ps[:],
                        lhsT=wl8_w[:, 2 * j : 2 * j + 2, :],
                        rhs=x8_t[:, 2 * j : 2 * j + 2, :],
                        start=False, stop=(j == KT // 2 - 1),
                        perf_mode=mybir.MatmulPerfMode.DoubleRow,
                    )
                return ps

            # Two 8-tile phases (8 mains -> 8 corrs, twice): only 3
            # fp16<->fp8 PE weight-pipeline switches in the whole kernel,
            # and the early PE stream is all main chains (whose W deps are
            # small and land first), so the DMA-paced head keeps the PE
            # busy. All 8 PSUM banks hold one phase's accumulators; the
            # second-half W tiles stream in as the first half's are freed.
            pss = {}
            H = MT // 2
            for t in range(H):
                main_chain(t)
                fetch_w_plane(t + H, 0)
            for j in range(H):
                ps = corr_chain(j)
                fetch_w_plane(j + H, 1)
                fetch_w_plane(j + H, 2)
                epilogue(j, ps)
            for t in range(H, MT):
                main_chain(t)
            for j in range(H, MT):
                epilogue(j, corr_chain(j), last=(j == MT - 1))

    nc.compile()
    return nc


def _relayout_w(w):
    """[N, M] -> w_dev[t*128 + p, kb*128 + m] = w[kb*128 + p, t*128 + m]
    so each m-tile's [128, N] slab is row-contiguous."""
    return np.ascontiguousarray(
        w.reshape(KT, 128, MT, 128).transpose(2, 1, 0, 3).reshape(M, N)
    )


def host_prep(x, eternal_weights, classical_weights, classical_biases):
    """Shard + lay out the inputs for the 8 cores (DMA-friendly layouts)."""
    x = np.ascontiguousarray(x, dtype=np.float32)
    w = np.ascontiguousarray(classical_weights, dtype=np.float32)
    cb = np.asarray(classical_biases, dtype=np.float32)

    xh16 = x.astype(np.float16)
    x8 = x.astype(NP_F8)
    xl8 = ((x - xh16.astype(np.float32)) * CSCALE).astype(NP_F8)
    wh16 = w.astype(np.float16)
    w8 = w.astype(NP_F8)
    wl8 = ((w - wh16.astype(np.float32)) * CSCALE).astype(NP_F8)
    # power-of-2 prescale (exact in fp16) so the fp16 main pass lands in
    # PSUM at the same 2^CSH fixed-point scale as the fp8 corrections
    xh = xh16 * np.float16(32.0)
    wh = wh16 * np.float16(64.0)

    wh_dev = _relayout_w(wh)
    w8_dev = _relayout_w(w8)
    wl8_dev = _relayout_w(wl8)

    # angles actually used: [D, M, 3] -> [27, M]; device layout
    # ang[p, g*MT + t] = angle_g[t*128 + p], zero-padded to GPAD slots.
    a = np.transpose(np.asarray(eternal_weights[:, :M, :3], dtype=np.float32),
                     (0, 2, 1)).reshape(NGATE, M)
    ar = a.reshape(NGATE, MT, 128)  # [g, t, p]
    ang = np.zeros((128, GPAD, MT), dtype=np.float32)
    ang[:, :NGATE, :] = np.transpose(ar, (2, 0, 1))
    ang = np.ascontiguousarray(ang.reshape(128, GPAD * MT))

    cbt = np.ascontiguousarray(cb.reshape(MT, 128).T)  # [128, MT]

    def shard_xt(xs):
        # [BS, N] -> [128, KT*BS]: xt[p, kb*BS + b] = xs[b, kb*128 + p]
        return np.ascontiguousarray(
            xs.reshape(BS, KT, 128).transpose(2, 1, 0).reshape(128, KT * BS)
        )

    in_maps = []
    for c in range(N_CORES):
        sl = slice(c * BS, (c + 1) * BS)
        in_maps.append({
            "xh": shard_xt(xh[sl]),
            "x8": shard_xt(x8[sl]),
            "xl8": shard_xt(xl8[sl]),
            "wh": wh_dev, "w8": w8_dev, "wl8": wl8_dev,
            "ang": ang, "cbt": cbt,
        })
    return in_maps


def host_post(results):
    """Reassemble [4096, 2048] from the 8 cores' out_dev blocks."""
    parts = []
    for c in range(N_CORES):
        od = np.asarray(results[c]["out_dev"], dtype=np.float32)  # [MT, 128, BS]
        # outT[t*128 + ml, b] = od[t, ml, b]
        parts.append(od.reshape(M, BS).T)  # [BS, M]
    return np.ascontiguousarray(np.concatenate(parts, axis=0), dtype=np.float32)


_NC_CACHE = {}


def _get_program():
    if "nc" not in _NC_CACHE:
        _NC_CACHE["nc"] = build_program()
    return _NC_CACHE["nc"]


def kernel(x, eternal_weights, eternal_biases, classical_weights, classical_biases,
           _trace=False):
    nc = _get_program()
    in_maps = host_prep(x, eternal_weights, classical_weights, classical_biases)
    res = run_bass_kernel_spmd(nc, in_maps, list(range(N_CORES)), trace=_trace)
    out = host_post(res.results)
    if _trace:
        kernel.last_exec_time_ns = res.exec_time_ns
        kernel.last_results = res
    return out


# revision 41
# speedup vs baseline: 1.0060x; 1.0060x over previous
"""Trainium2 Bass kernel for nn_EternalNeuralLayer.

Math: out = tanh(x @ W_c + b_c + probs[None, :]) where
probs[j] = |state[j, 0]|^2 after 27 nearest-neighbour circulant "gates"
applied to the uniform state 1/sqrt(n). Each gate matrix
G = cos*I - sin*P + sin*P^T is circulant, and the uniform vector is its
eigenvector with eigenvalue cos(theta), so the state stays uniform:
probs[j] = (prod_{d,g} cos(ew[d, j, g]))^2 / n   (g in 0..2, d in 0..8).

Sharding: data-parallel over the batch (8 cores x 512 rows). Every core
streams the full classical_weights [2048, 2048], computes its x-shard's
GEMM as outT[m, b] = sum_k W[k, m] * xT[k, b] (output m-on-partitions),
and writes its outT shard. The eternal-probs product is computed
on-device per core from the [27, 2048] angle slice (tiny). No
collectives needed.

GEMM precision scheme (~16 effective mantissa bits, abs err ~2e-3):
  z * 2^11 = (xh*2^5) @ (Wh*2^6)  fp16 x fp16 matmul   (1.0 cyc/row)
           + xl8 @ W8             fp8e4 DoubleRow      (0.5 cyc/row)
           + x8  @ Wl8            fp8e4 DoubleRow      (0.5 cyc/row)
  xh = fp16(x), xl8 = e4m3((x - xh) * 2^11), x8 = e4m3(x); same for W.
The fp16 main pass carries 11+11 bits; the two fp8 DoubleRow passes
(2 k-rows/cycle) correct each operand's fp16 rounding to ~16+16 bits.
Cross-term quantization error ~9e-4 rms on the preactivation. This
replaces the 3x fp32r hi/lo scheme: 2.0 pass-equivalents instead of
3.0, and 25MB HBM traffic instead of 46MB per core.

The power-of-2 operand prescale puts the fp16 main pass at the same
2^11 fixed-point scale the fp8 correction planes already carry, so all
three passes accumulate into ONE PSUM bank and the epilogue is a single
Scalar-engine tanh(psum * 2^-11 + bias) per m-tile.
"""

import math
import os
import sys

import numpy as np

for _p in ("/opt/trn_rl_repo", "/root/.axon_site/_ro/trn_rl_repo"):
    if _p not in sys.path and os.path.isdir(_p):
        sys.path.append(_p)

import concourse.bass as bass  # noqa: E402
import concourse.tile as tile  # noqa: E402
from concourse import bacc, mybir  # noqa: E402
from concourse.bass_utils import run_bass_kernel_spmd  # noqa: E402

N_CORES = 8
B, N, M, D = 4096, 2048, 2048, 9
BS = B // N_CORES  # 512 batch rows per core
KT = N // 128  # 16 contraction tiles
MT = M // 128  # 16 output m-tiles
WPRE = 8  # W-tile pool depth (per plane: one 8-tile phase in flight)
NGATE = D * 3  # 27 rotation gates
GPAD = 32  # padded gate slots (pad with 0.0 -> cos = 1)
CSH = 11  # correction planes carry 2^CSH fixed-point scale
CSCALE = float(2.0**CSH)

F32 = mybir.dt.float32
F16 = mybir.dt.float16
F8 = mybir.dt.float8e4
NP_F8 = mybir.dt.np(mybir.dt.float8e4)  # ml_dtypes.float8_e4m3

XC = 2  # xh DMA chunk: 2 k-slabs (2KB rows)
X8C = 4  # x8/xl8 DMA chunk: 4 k-slabs (2KB rows)


def build_program():
    nc = bacc.Bacc(
        "TRN2", target_bir_lowering=False, debug=False, num_devices=N_CORES
    )
    # x planes: [p, kb*BS + b] = plane[b, kb*128 + p]
    xh_d = nc.dram_tensor("xh", [128, KT * BS], F16, kind="ExternalInput").ap()
    x8_d = nc.dram_tensor("x8", [128, KT * BS], F8, kind="ExternalInput").ap()
    xl8_d = nc.dram_tensor("xl8", [128, KT * BS], F8, kind="ExternalInput").ap()
    # W planes: [t*128 + p, kb*128 + m] = Wplane[kb*128 + p, t*128 + m]
    wh_d = nc.dram_tensor("wh", [M, N], F16, kind="ExternalInput").ap()
    w8_d = nc.dram_tensor("w8", [M, N], F8, kind="ExternalInput").ap()
    wl8_d = nc.dram_tensor("wl8", [M, N], F8, kind="ExternalInput").ap()
    ang_d = nc.dram_tensor("ang", [128, GPAD * MT], F32, kind="ExternalInput").ap()
    cbt_d = nc.dram_tensor("cbt", [128, MT], F32, kind="ExternalInput").ap()
    # out_dev[t, ml, b] = tanh(...)[m = t*128 + ml, b]
    out_d = nc.dram_tensor(
        "out_dev", [MT, 128, BS], F32, kind="ExternalOutput"
    ).ap()

    with tile.TileContext(nc) as tc:
        with (
            tc.tile_pool(name="xt", bufs=1) as xt_pool,
            tc.tile_pool(name="w", bufs=WPRE) as w_pool,
            tc.tile_pool(name="ps", bufs=3, space="PSUM") as ps_pool,
            tc.tile_pool(name="out", bufs=3) as out_pool,
            tc.tile_pool(name="small", bufs=1) as small_pool,
        ):
            # --- resident x planes (filled by chunked DMAs; chunk sizes
            # keep every DMA row >= 2KB contiguous) ---
            xh_t = xt_pool.tile([128, KT, BS], F16, tag="xh")
            x8_t = xt_pool.tile([128, KT, BS], F8, tag="x8")
            xl8_t = xt_pool.tile([128, KT, BS], F8, tag="xl8")

            def fetch_xh_slabs(lo, hi):  # k-slab range [lo, hi)
                nc.sync.dma_start(
                    xh_t[:, lo:hi, :], xh_d[:, lo * BS : hi * BS]
                )

            def fetch_xh(c):  # chunk of XC k-slabs
                fetch_xh_slabs(c * XC, (c + 1) * XC)

            def fetch_x8(c, t, d):  # chunk of X8C k-slabs
                nc.sync.dma_start(
                    t[:, c * X8C : (c + 1) * X8C, :],
                    d[:, c * X8C * BS : (c + 1) * X8C * BS],
                )

            # --- GEMM input DMAs on the sync HWDGE ring; output stores use
            # the scalar ring so a store waiting on ACT never head-of-line
            # blocks W loads. ---
            wts = {}

            def w_plane_parts(t, plane, spans=((0, KT),)):
                """Allocate the plane tile, return per-span DMA thunks
                (spans in k-slab units)."""
                tag, dt_, d = (
                    ("wh", F16, wh_d), ("w8", F8, w8_d), ("wl8", F8, wl8_d)
                )[plane]
                if plane == 0:
                    wt = w_pool.tile([128, KT * 128], dt_, name="wt", tag=tag)
                else:
                    wt = w_pool.tile([128, KT, 128], dt_, name="wt", tag=tag)
                rows = slice(t * 128, (t + 1) * 128)
                wts.setdefault(t, []).append(wt)

                def issue(lo, hi):
                    nc.sync.dma_start(
                        wt[:, lo:hi, :]
                        if plane else wt[:, lo * 128 : hi * 128],
                        d[rows, lo * 128 : hi * 128],
                    )
                return [lambda lo=lo, hi=hi: issue(lo, hi) for lo, hi in spans]

            def fetch_w_plane(t, plane):
                w_plane_parts(t, plane)[0]()

            # --- eternal probs bias inputs ride the (idle until stores)
            # scalar ring so bias_t is ready long before the first epilogue
            ang = small_pool.tile([128, GPAD * MT], F32)
            nc.scalar.dma_start(ang[:], ang_d[:])
            cbt = small_pool.tile([128, MT], F32)
            nc.scalar.dma_start(cbt[:], cbt_d[:])

            # Issue order. Sync-queue DMA issue costs ~650ns/descriptor, so
            # the head uses few, ~0.25-0.5MB descriptors, ordered to match
            # the PE consumption schedule (mains 0-3, then corr0 lagging):
            # xh chunks with wh0's two halves woven in, wh1-3, the fp8
            # planes for the first corrs, then the steady pipeline.
            # ~0.25MB head descriptors: finer slicing starts the PE earlier
            # but into an unramped clock + stop-start stream and measures
            # net-worse; this pacing keeps the PE stall-free from its start
            wh0_parts = w_plane_parts(0, 0, spans=((0, 8), (8, 16)))
            fetch_xh_slabs(0, 2)
            wh0_parts[0]()
            fetch_xh(1)
            fetch_xh(2)
            wh0_parts[1]()
            for c in range(3, KT // XC):
                fetch_xh(c)
            fetch_w_plane(1, 0)
            fetch_w_plane(2, 0)
            fetch_w_plane(3, 0)
            for c in range(KT // X8C):
                fetch_x8(c, xl8_t, xl8_d)
            fetch_w_plane(0, 1)
            fetch_w_plane(0, 2)
            for c in range(KT // X8C):
                fetch_x8(c, x8_t, x8_d)
            for t in range(4, 8):
                fetch_w_plane(t, 0)
            for t in range(1, 8):
                fetch_w_plane(t, 1)
                fetch_w_plane(t, 2)

            # explicit zeros AP as ACT bias: a float bias would materialize
            # a const AP, whose TENSOR_LOAD serializes into the preamble
            zeros = small_pool.tile([128, 1], F32)
            nc.gpsimd.memset(zeros[:], 0.0)

            cosa = small_pool.tile([128, GPAD * MT], F32)
            # cos(a) = sin(a + pi/2); wrap into ACT Sin's [-pi, pi] domain
            # (|a| < 3pi/2 + pi holds for randn angles).
            nc.vector.add_range_wrap(
                cosa[:], ang[:], shift=math.pi / 2, bound=math.pi,
                period=2 * math.pi,
            )
            nc.scalar.activation(
                cosa[:], cosa[:], mybir.ActivationFunctionType.Sin,
                bias=zeros[:],
            )
            # tree-product over the 32 gate slots -> [128, MT]
            half = GPAD * MT // 2
            while half >= MT:
                nc.vector.tensor_mul(
                    cosa[:, 0:half], cosa[:, 0:half], cosa[:, half : 2 * half]
                )
                half //= 2
            bias_t = small_pool.tile([128, MT], F32)
            # probs = (prod cos)^2 / n
            nc.scalar.activation(
                bias_t[:],
                cosa[:, 0:MT],
                mybir.ActivationFunctionType.Square,
                bias=zeros[:],
                scale=1.0 / math.sqrt(N),
            )
            nc.vector.tensor_add(bias_t[:], bias_t[:], cbt[:])

            # --- column-parallel GEMM over 16 m-tiles ---
            def epilogue(t, ps, last=False):
                ot = out_pool.tile([128, BS], F32, name="ot", tag="ot", bufs=4)
                # out = tanh(ps * 2^-CSH + bias)  (single ACT pass)
                if not last:
                    nc.scalar.activation(
                        ot[:],
                        ps[:],
                        mybir.ActivationFunctionType.Tanh,
                        bias=bias_t[:, t : t + 1],
                        scale=1.0 / CSCALE,
                    )
                    nc.scalar.dma_start(out_d[t], ot[:])
                    return
                # final tile: halves, stores split across both rings, so
                # the exposed tail is ACT/2 + store/2 instead of ACT+store
                h = BS // 2
                nc.scalar.activation(
                    ot[:, 0:h], ps[:, 0:h],
                    mybir.ActivationFunctionType.Tanh,
                    bias=bias_t[:, t : t + 1], scale=1.0 / CSCALE,
                )
                nc.sync.dma_start(out_d[t][:, 0:h], ot[:, 0:h])
                nc.scalar.activation(
                    ot[:, h:BS], ps[:, h:BS],
                    mybir.ActivationFunctionType.Tanh,
                    bias=bias_t[:, t : t + 1], scale=1.0 / CSCALE,
                )
                nc.scalar.dma_start(out_d[t][:, h:BS], ot[:, h:BS])

            def main_chain(t):
                # fp16 main pass (prescaled by 2^11 across the operands)
                wh_w = wts[t][0]
                ps = ps_pool.tile([128, BS], F32, name="ps", tag="ps", bufs=8)
                pss[t] = ps
                for kb in range(KT):
                    nc.tensor.matmul(
                        ps[:],
                        lhsT=wh_w[:, kb * 128 : (kb + 1) * 128],
                        rhs=xh_t[:, kb, :],
                        start=(kb == 0), stop=False,
                    )

            def corr_chain(t):
                # fp8 DoubleRow corrections, 2 k-slabs per matmul, same
                # 2^11 scale -> same PSUM accumulator as the main pass
                _, w8_w, wl8_w = wts.pop(t)
                ps = pss.pop(t)
                for j in range(KT // 2):
                    nc.tensor.matmul(
                        ps[:],
                        lhsT=w8_w[:, 2 * j : 2 * j + 2, :],
                        rhs=xl8_t[:, 2 * j : 2 * j + 2, :],
                        start=False, stop=False,
                        perf_mode=mybir.MatmulPerfMode.DoubleRow,
                    )
                for j in range(KT // 2):
                    nc.tensor.matmul(
                        ps[:],
                        lhsT=wl8_w[:, 2 * j : 2 * j + 2, :],
                        rhs=x8_t[:, 2 * j : 2 * j + 2, :],
                        start=False, stop=(j == KT // 2 - 1),
                        perf_mode=mybir.MatmulPerfMode.DoubleRow,
                    )
                return ps

            # Two 8-tile phases (8 mains -> 8 corrs, twice): only 3
            # fp16<->fp8 PE weight-pipeline switches in the whole kernel,
            # and the early PE stream is all main chains (whose W deps are
            # small and land first), so the DMA-paced head keeps the PE
            # busy. All 8 PSUM banks hold one phase's accumulators; the
            # second-half W tiles stream in as the first half's are freed.
            pss = {}
            H = MT // 2
            for t in range(H):
                main_chain(t)
                fetch_w_plane(t + H, 0)
            for j in range(H):
                ps = corr_chain(j)
                fetch_w_plane(j + H, 1)
                fetch_w_plane(j + H, 2)
                epilogue(j, ps)
            for t in range(H, MT):
                main_chain(t)
            for j in range(H, MT):
                epilogue(j, corr_chain(j), last=(j == MT - 1))

    nc.compile()
    return nc


def _relayout_w(w):
    """[N, M] -> w_dev[t*128 + p, kb*128 + m] = w[kb*128 + p, t*128 + m]
    so each m-tile's [128, N] slab is row-contiguous."""
    return np.ascontiguousarray(
        w.reshape(KT, 128, MT, 128).transpose(2, 1, 0, 3).reshape(M, N)
    )


def host_prep(x, eternal_weights, classical_weights, classical_biases):
    """Shard + lay out the inputs for the 8 cores (DMA-friendly layouts)."""
    x = np.ascontiguousarray(x, dtype=np.float32)
    w = np.ascontiguousarray(classical_weights, dtype=np.float32)
    cb = np.asarray(classical_biases, dtype=np.float32)

    xh16 = x.astype(np.float16)
    x8 = x.astype(NP_F8)
    xl8 = ((x - xh16.astype(np.float32)) * CSCALE).astype(NP_F8)
    wh16 = w.astype(np.float16)
    w8 = w.astype(NP_F8)
    wl8 = ((w - wh16.astype(np.float32)) * CSCALE).astype(NP_F8)
    # power-of-2 prescale (exact in fp16) so the fp16 main pass lands in
    # PSUM at the same 2^CSH fixed-point scale as the fp8 corrections
    xh = xh16 * np.float16(32.0)
    wh = wh16 * np.float16(64.0)

    wh_dev = _relayout_w(wh)
    w8_dev = _relayout_w(w8)
    wl8_dev = _relayout_w(wl8)

    # angles actually used: [D, M, 3] -> [27, M]; device layout
    # ang[p, g*MT + t] = angle_g[t*128 + p], zero-padded to GPAD slots.
    a = np.transpose(np.asarray(eternal_weights[:, :M, :3], dtype=np.float32),
                     (0, 2, 1)).reshape(NGATE, M)
    ar = a.reshape(NGATE, MT, 128)  # [g, t, p]
    ang = np.zeros((128, GPAD, MT), dtype=np.float32)
    ang[:, :NGATE, :] = np.transpose(ar, (2, 0, 1))
    ang = np.ascontiguousarray(ang.reshape(128, GPAD * MT))

    cbt = np.ascontiguousarray(cb.reshape(MT, 128).T)  # [128, MT]

    def shard_xt(xs):
        # [BS, N] -> [128, KT*BS]: xt[p, kb*BS + b] = xs[b, kb*128 + p]
        return np.ascontiguousarray(
            xs.reshape(BS, KT, 128).transpose(2, 1, 0).reshape(128, KT * BS)
        )

    in_maps = []
    for c in range(N_CORES):
        sl = slice(c * BS, (c + 1) * BS)
        in_maps.append({
            "xh": shard_xt(xh[sl]),
            "x8": shard_xt(x8[sl]),
            "xl8": shard_xt(xl8[sl]),
            "wh": wh_dev, "w8": w8_dev, "wl8": wl8_dev,
            "ang": ang, "cbt": cbt,
        })
    return in_maps


def host_post(results):
    """Reassemble [4096, 2048] from the 8 cores' out_dev blocks."""
    parts = []
    for c in range(N_CORES):
        od = results[c]["out_dev"]  # [MT, 128, BS]
        # outT[t*128 + ml, b] = od[t, ml, b]
        parts.append(od.reshape(M, BS).T)  # [BS, M]
    return np.ascontiguousarray(np.concatenate(parts, axis=0), dtype=np.float32)


_NC_CACHE = {}


def _get_program():
    if "nc" not in _NC_CACHE:
        _NC_CACHE["nc"] = build_program()
    return _NC_CACHE["nc"]


def kernel(x, eternal_weights, eternal_biases, classical_weights, classical_biases,
           _trace=False):
    nc = _get_program()
    in_maps = host_prep(x, eternal_weights, classical_weights, classical_biases)
    res = run_bass_kernel_spmd(nc, in_maps, list(range(N_CORES)), trace=_trace)
    out = host_post(res.results)
    if _trace:
        kernel.last_exec_time_ns = res.exec_time_ns
        kernel.last_results = res
    return out
